# revision 24
# baseline (speedup 1.0000x reference)
"""Trainium2 Bass kernel for BaseLUTLayer (probabilistic LUT node eval).

Math (per reference):
  x_eff = where(flip, 1 - x, x)                      # (B, IN)
  g[b,n,j] = x_eff[b, mapping[n,j]]                  # gather, (B, N, 6)
  out[b,n] = sum_k sigmoid(lut[n,k]) * prod_j (g_j if bit_j(k) else 1-g_j)

Evaluated on-device as a 6-level multilinear contraction per (b, n):
  level 0 folds the LSB of the 64-entry sigmoid table with per-node
  scalars (tensor_scalar FMA, per-partition scalar operands), levels 1-5
  are lerps V = U_even + a_j * (U_odd - U_even) done with tensor_tensor
  ops and a 0-stride broadcast AP for a_j.

Flip is carried in the fp16 sign bit (host packs xs = flip ? -x : x,
a pure bit-OR), reconstructed on device as a = (xs_u16 >> 15) + xs,
exact because sign(-0.0) still flags the flip.  Gather rows shrink to
512 B (fp16 only).

Engine assignment follows the TRN2 cost model: DVE (0.52 cyc/elem
tensor_tensor fp16, 0.25 tensor_scalar) runs the lerp tree, ACT
(1.2 GHz, scalar FMA via activation) takes most of level 0 + sigmoid,
Pool (0.42-0.6 efficiency) takes the rest of level 0, the sign shift,
and the smallest level.

Sharding: nodes split 8 ways (1024 nodes/core); batch replicated.
Per-core output is (1024, 256) fp32, host concatenates + transposes.
"""

import numpy as np

B = 256
IN = 8192
NN = 8192
FAN = 6
NPAT = 64
NCORES = 8
PT = 128  # nodes per tile (partition dim)

_CACHE = {}


def _build_nc(nl, b, inp):
    """Build + compile the SPMD Bass program for one core's slice.

    nl: nodes per core, b: batch (replicated), inp: input size.
    """
    import concourse.bacc as bacc
    import concourse.mybir as mybir
    from concourse.tile import TileContext
    from concourse._compat import get_trn_type

    dt = mybir.dt
    Alu = mybir.AluOpType
    Act = mybir.ActivationFunctionType

    nt = nl // PT
    n_idx_t = PT * FAN        # gather indices per tile (768)
    iw = nl * FAN // 16       # idx wrap columns

    nc = bacc.Bacc(
        get_trn_type() or "TRN2",
        target_bir_lowering=False,
        debug=False,
        num_devices=NCORES,
    )
    rowb = 2 * b              # gather row: b fp16 sign-packed values
    xsT = nc.dram_tensor("xsT", [inp, rowb], dt.uint8, kind="ExternalInput")
    lut = nc.dram_tensor("lut", [nl, NPAT], dt.float32, kind="ExternalInput")
    idx = nc.dram_tensor("idx", [128, iw], dt.int16, kind="ExternalInput")
    outT = nc.dram_tensor("outT", [nl, b], dt.float32, kind="ExternalOutput")

    cdt = dt.float16

    with TileContext(nc) as tc:
        with (
            tc.tile_pool(name="const", bufs=1) as cpool,
            tc.tile_pool(name="ld", bufs=2) as ld,
            tc.tile_pool(name="small", bufs=3) as sm,
            tc.tile_pool(name="abp", bufs=3) as abp,
            tc.tile_pool(name="work", bufs=3) as wk,
        ):
            idx_sb = cpool.tile([128, iw], dt.int16)
            nc.sync.dma_start(idx_sb[:, :], idx[:, :])

            def prologue(t):
                # --- loads: gather brings sign-packed fp16 x rows ---
                g = ld.tile([128, FAN, rowb], dt.uint8, tag="g")
                nc.gpsimd.dma_gather(
                    g[:, :, :], xsT[:, :],
                    idx_sb[:, t * (n_idx_t // 16):(t + 1) * (n_idx_t // 16)],
                    n_idx_t, n_idx_t, rowb,
                )
                xs = g[:, :, :].bitcast(cdt)          # [128, FAN, b] fp16
                xs_i = g[:, :, :].bitcast(dt.uint16)  # same bits as uint16
                lut_t = ld.tile([128, NPAT], dt.float32, tag="lut")
                nc.sync.dma_start(lut_t[:, :], lut[t * PT:(t + 1) * PT, :])

                # --- per-node table prep (Moebius coefficients for a
                # combined fold of levels 0..2) ---
                # sig[k] = sigmoid(lut[k]); with S = sig[8q + i]:
                #   V3[q] = A[q] + a1*B[q] + a2*C[q] + a1*a2*D[q]
                #   A = S0 + a0*(S1-S0)
                #   B = (S2-S0) + a0*((S3-S2)-(S1-S0))
                #   C = (S4-S0) + a0*((S5-S4)-(S1-S0))
                #   D = (S6-S4-S2+S0) + a0*(((S7-S6)-(S5-S4))-((S3-S2)-(S1-S0)))
                # All scale/bias pairs are strided views of difference chains:
                #   d0[m]  = sig[2m+1]-sig[2m]            (32)
                #   dE[m]  = sig[4m+2]-sig[4m]            (16)
                #   dD[m]  = d0[2m+1]-d0[2m]              (16)
                #   dF[q]  = sig[8q+4]-sig[8q]            (8)
                #   dG[q]  = d0[4q+2]-d0[4q]              (8)
                #   dH[q]  = dE[2q+1]-dE[2q]              (8)
                #   dI[q]  = dD[2q+1]-dD[2q]              (8)
                # giving (bias, scale): A=(S0=sig[0::8], d0[0::4]),
                # B=(dE[0::2], dD[0::2]), C=(dF, dG), D=(dH, dI).
                sig = sm.tile([128, NPAT], dt.float32, tag="sig")
                nc.scalar.activation(sig[:, :], lut_t[:, :], Act.Sigmoid)
                d0 = sm.tile([128, NPAT // 2], dt.float32, tag="d0")
                nc.vector.tensor_sub(d0[:, :], sig[:, 1::2], sig[:, 0::2])
                dE = sm.tile([128, NPAT // 4], dt.float32, tag="dE")
                nc.vector.tensor_sub(dE[:, :], sig[:, 2::4], sig[:, 0::4])
                dD = sm.tile([128, NPAT // 4], dt.float32, tag="dD")
                nc.gpsimd.tensor_sub(dD[:, :], d0[:, 1::2], d0[:, 0::2])
                dF = sm.tile([128, 8], dt.float32, tag="dF")
                nc.vector.tensor_sub(dF[:, :], sig[:, 4::8], sig[:, 0::8])
                dG = sm.tile([128, 8], dt.float32, tag="dG")
                nc.vector.tensor_sub(dG[:, :], d0[:, 2::4], d0[:, 0::4])
                dH = sm.tile([128, 8], dt.float32, tag="dH")
                nc.gpsimd.tensor_sub(dH[:, :], dE[:, 1::2], dE[:, 0::2])
                dI = sm.tile([128, 8], dt.float32, tag="dI")
                nc.gpsimd.tensor_sub(dI[:, :], dD[:, 1::2], dD[:, 0::2])

                # --- flip fixup: a = (xs_u16 >> 15) + xs ---
                # fanin 0 first (short critical path into the TS layer),
                # then 1,2 (needed for the Moebius combine), then 3-5.
                c = sm.tile([128, FAN, b], dt.uint16, tag="c")
                a = sm.tile([128, FAN, b], cdt, tag="a")
                nc.vector.tensor_scalar(
                    out=c[:, 0:1, :], in0=xs_i[:, 0:1, :], scalar1=15,
                    scalar2=None, op0=Alu.logical_shift_right,
                )
                nc.vector.tensor_add(a[:, 0, :], c[:, 0, :], xs[:, 0, :])
                nc.vector.tensor_scalar(
                    out=c[:, 1:3, :], in0=xs_i[:, 1:3, :], scalar1=15,
                    scalar2=None, op0=Alu.logical_shift_right,
                )
                nc.vector.tensor_add(a[:, 1:3, :], c[:, 1:3, :], xs[:, 1:3, :])
                g12 = sm.tile([128, 1, b], cdt, tag="g12")
                nc.vector.tensor_mul(g12[:, 0, :], a[:, 1, :], a[:, 2, :])
                nc.vector.tensor_scalar(
                    out=c[:, 3:, :], in0=xs_i[:, 3:, :], scalar1=15,
                    scalar2=None, op0=Alu.logical_shift_right,
                )
                nc.vector.tensor_add(a[:, 3:, :], c[:, 3:, :], xs[:, 3:, :])

                # --- TS layer: 32 per-partition-scalar FMAs in a0,
                # producing A,B,C,D [128, 8, b]; emitted here (still in the
                # prologue) so ACT/Pool run tile t's layer while DVE chews
                # tile t-1's tree. ---
                a0 = a[:, 0, :]
                AB = abp.tile([128, 4, 8, b], cdt, tag="AB")  # [ABCD, q, b]
                coef = [
                    (sig[:, 0::8], d0[:, 0::4]),
                    (dE[:, 0::2], dD[:, 0::2]),
                    (dF[:, :], dG[:, :]),
                    (dH[:, :], dI[:, :]),
                ]
                if t == 0:
                    homes = ["dve", "act", "dve", "pool", "dve", "act", "act", "pool"] * 2
                elif t == 1:
                    homes = ["act", "act", "dve", "pool"] * 4
                else:
                    homes = ["act", "act", "act", "pool"] * 4
                k = 0
                for q in range(8):
                    for ci in range(4):
                        bias, scale = coef[ci]
                        dst = AB[:, ci, q, :]
                        sc = scale[:, q:q + 1]
                        bi = bias[:, q:q + 1]
                        h = homes[k % 16]
                        if h == "pool":
                            nc.gpsimd.tensor_scalar(
                                out=dst, in0=a0, scalar1=sc, scalar2=bi,
                                op0=Alu.mult, op1=Alu.add,
                            )
                        elif h == "dve":
                            nc.vector.tensor_scalar(
                                out=dst, in0=a0, scalar1=sc, scalar2=bi,
                                op0=Alu.mult, op1=Alu.add,
                            )
                        else:
                            nc.scalar.activation(
                                dst, a0, Act.Identity, scale=sc, bias=bi)
                        k += 1
                return {"AB": AB, "a": a, "g12": g12}

            def body(t, ctx):
                """Generator: yields between op clusters so two tail tiles
                can interleave on the in-order engine queues."""
                AB, a, g12 = ctx["AB"], ctx["a"], ctx["g12"]
                V = wk.tile([128, 8, b], cdt, tag="V3")
                PB = wk.tile([128, 2, 4, b], cdt, tag="PB")
                PC = wk.tile([128, 2, 4, b], cdt, tag="PC")
                PD = wk.tile([128, 2, 4, b], cdt, tag="PD")
                t1 = wk.tile([128, 2, 4, b], cdt, tag="t1")
                t2 = wk.tile([128, 2, 4, b], cdt, tag="t2")
                for half in range(2):
                    qs = slice(half * 4, half * 4 + 4)
                    a1 = a[:, 1:2, :].broadcast_to([128, 4, b])
                    a2 = a[:, 2:3, :].broadcast_to([128, 4, b])
                    g12b = g12[:, :, :].broadcast_to([128, 4, b])
                    nc.vector.tensor_mul(PB[:, half], AB[:, 1, qs, :], a1)
                    nc.vector.tensor_mul(PC[:, half], AB[:, 2, qs, :], a2)
                    nc.vector.tensor_mul(PD[:, half], AB[:, 3, qs, :], g12b)
                    yield
                    nc.vector.tensor_add(t1[:, half], AB[:, 0, qs, :], PB[:, half])
                    nc.vector.tensor_add(t2[:, half], PC[:, half], PD[:, half])
                    nc.vector.tensor_add(V[:, qs, :], t1[:, half], t2[:, half])
                    yield

                # --- levels 3..5: V = U_e + a_j*(U_o - U_e) ---
                for j in range(3, 6):
                    h = 32 >> j  # output pattern count (4, 2, 1)
                    eng = nc.gpsimd if j >= 4 and t != nt - 1 else nc.vector
                    D = wk.tile([128, h, b], cdt, tag=f"D{j}")
                    eng.tensor_sub(D[:, :, :], V[:, 1::2, :], V[:, 0::2, :])
                    yield
                    aj = a[:, j:j + 1, :].broadcast_to([128, h, b])
                    P = wk.tile([128, h, b], cdt, tag=f"P{j}")
                    eng.tensor_mul(P[:, :, :], D[:, :, :], aj)
                    yield
                    odt = dt.float32 if j == 5 else cdt
                    Vn = wk.tile([128, h, b], odt, tag=f"V{j}")
                    eng.tensor_add(Vn[:, :, :], P[:, :, :], V[:, 0::2, :])
                    V = Vn
                    yield

                nc.sync.dma_start(outT[t * PT:(t + 1) * PT, :], V[:, 0, :])

            def drain(gens):
                gens = list(gens)
                while gens:
                    for gn in list(gens):
                        try:
                            next(gn)
                        except StopIteration:
                            gens.remove(gn)

            # software pipeline: tile t+1's prologue (gather + fixup +
            # TS layer on ACT/Pool) is emitted before tile t's DVE tree.
            # The last two bodies are interleaved to fill dependency-chain
            # bubbles once no prologue work remains.
            ctx = prologue(0)
            for t in range(nt - 2):
                nxt = prologue(t + 1)
                drain([body(t, ctx)])
                ctx = nxt
            ctx2 = prologue(nt - 1)
            drain([body(nt - 2, ctx), body(nt - 1, ctx2)])

    nc.compile()
    return nc


def _prep_core_inputs(x, lut_table, mapping, flip_mask, nl, b, inp, n_cores=NCORES):
    """Host-side layout prep (pure data movement): transpose + slice + index
    pack + sign-bit OR (flip packed into the fp16 sign)."""
    x16 = np.ascontiguousarray(x.T, dtype=np.float16)              # (IN, B)
    fT = np.asarray(flip_mask).T                                   # (IN, B) bool
    xs = (x16.view(np.uint16) | (fT.astype(np.uint16) << 15)).view(np.float16)
    xs_u8 = np.ascontiguousarray(xs).view(np.uint8)                # (IN, 2B)
    nt = nl // PT
    in_maps = []
    for c in range(n_cores):
        sl = slice(c * nl, (c + 1) * nl)
        lut_c = np.ascontiguousarray(lut_table[sl], dtype=np.float32)
        m_c = np.asarray(mapping[sl])                              # (nl, 6) int32
        # gather order: j = (t*6+f)*128 + p  ->  m_c[t*128+p, f]
        order = m_c.reshape(nt, PT, FAN).transpose(0, 2, 1).reshape(-1)
        idx16 = order.astype(np.int16)
        wrapped = np.ascontiguousarray(idx16.reshape(-1, 16).T)    # (16, nl*6/16)
        idx_full = np.tile(wrapped, (8, 1))                        # (128, ...)
        in_maps.append({"xsT": xs_u8, "lut": lut_c, "idx": idx_full})
    return in_maps


def _run(nc, in_maps, **kw):
    from concourse.bass_utils import run_bass_kernel_spmd

    last = None
    for attempt in range(3):
        try:
            return run_bass_kernel_spmd(nc, in_maps, list(range(NCORES)), **kw)
        except Exception as e:  # transient device errors happen on this fabric
            last = e
            if "UNRECOVERABLE" not in str(e) and "UNAVAILABLE" not in str(e):
                raise
    raise last


def kernel(x, lut_table, mapping, flip_mask):
    b, inp = x.shape
    nn = lut_table.shape[0]
    nl = nn // NCORES
    key = (nl, b, inp)
    if key not in _CACHE:
        _CACHE[key] = _build_nc(nl, b, inp)
    nc = _CACHE[key]
    in_maps = _prep_core_inputs(x, lut_table, mapping, flip_mask, nl, b, inp)
    res = _run(nc, in_maps)
    outT = np.concatenate([res.results[c]["outT"] for c in range(NCORES)], axis=0)
    return np.ascontiguousarray(outT.T, dtype=np.float32)
